# revision 14
# baseline (speedup 1.0000x reference)
"""BertSelfAttention (ALiBi-style additive bias) on 8 TRN2 NeuronCores.

Problem: B=4, S=1024, D=1024, H=16 heads (HD=64), fp32.
  qkv = hidden @ Wqkv_w.T + Wqkv_b
  scores = q @ k.T / sqrt(64) + bias ;  probs = softmax(scores) ; out = probs @ v

Sharding: 8 cores = 4 batches x 2 head-groups. Core c handles batch c//2 and
heads [ (c%2)*8, (c%2)*8+8 ).  Per-core shards are prepared host-side in the
layouts the TensorEngine wants (contraction dim on partitions) and cast to
bf16, so every device DMA is a contiguous, full-rate read:
  hw  [D, S+1536]  = [hidden[b].T | Wqkv rows for this core, transposed]
  eb  [4,8,2,128,1024] = exp(bias^T) tiles packed per (head-pair, kc, q-half)
                     with the two heads of a pair side by side in columns

Device algorithm (per core), exploiting exp(s+b) = exp(s)*exp(b):
  - QKV projection as in the baseline (bf16 matmuls, fp32 PSUM), q rows
    pre-scaled by 1/8; the q/k rows of a head pair land on partitions
    0:64 / 64:128 of the qk tiles.
  - scores: per (pair, kc, q-half), TWO row-tiled K=64 matmuls run
    concurrently on the PE array (tile_position (0,0) and (64,0)), writing
    head0 -> cols 0:512, head1 -> cols 512:1024 of one [128,1024] PSUM tile.
  - exp on ScalarE (PSUM->SBUF bf16), then DVE multiply by the DMA'd
    exp(bias) tile (this replaces the baseline's identity-matmul bias add).
  - AV: per item, TWO col-tiled M=64 matmuls (tile_position (0,0)/(0,64))
    accumulate both heads' outT into one [128,1024] PSUM tile; TWO col-tiled
    M=1 ones-matmuls accumulate the softmax denominators.
  - normalize: reciprocal on DVE, partition-broadcast on GpSimd, bf16
    multiply on DVE, DMA out (host re-transposes).
QKV block emission is interleaved into the attention item loop (the filler
queue) so the in-order PE stream keeps the ScalarE exp pipeline fed.
PSUM budget (8 banks): scores 2 + AV 2 + denom 2 + qkv-proj 1 + v-proj 1.
"""

import numpy as np

import concourse.bacc as bacc
import concourse.bass as bass
import concourse.mybir as mybir
from concourse.tile import TileContext

B, S, D = 4, 1024, 1024
H = 16
HD = 64  # head dim
N_CORES = 8
HPC = 8  # heads per core
NPAIR = HPC // 2
OC = 3 * HPC * HD  # 1536 fused-qkv output rows per core
F32 = mybir.dt.float32
BF16 = mybir.dt.bfloat16

KC = S // 128  # 8 key-token chunks of 128
DC = D // 128  # 8 contraction chunks of 128
EB_PREFETCH = 11


def build_bass() -> bass.Bass:
    nc = bacc.Bacc()

    hw = nc.declare_dram_parameter("hw", [D, S + OC], BF16, isOutput=False)
    wb = nc.declare_dram_parameter("wb", [1, OC], BF16, isOutput=False)
    wbp = nc.declare_dram_parameter("wbp", [128, 12], F32, isOutput=False)
    eb = nc.declare_dram_parameter("eb", [NPAIR, KC, 2, 128, S], BF16, isOutput=False)
    oT = nc.declare_dram_parameter("oT", [HPC * HD, S], BF16, isOutput=True)

    with TileContext(nc) as tc:
        with (
            tc.tile_pool(name="const", bufs=1) as constp,
            tc.tile_pool(name="weights", bufs=1) as wp,
            tc.tile_pool(name="qk", bufs=1) as qkp,
            tc.tile_pool(name="vex", bufs=1) as vp,
            tc.tile_pool(name="ebias", bufs=16) as ebp,
            tc.tile_pool(name="exp", bufs=14) as ep,
            tc.tile_pool(name="outs", bufs=4) as op_,
            tc.tile_pool(name="ps_sc", bufs=1, space="PSUM") as ps_sc,
            tc.tile_pool(name="ps_av", bufs=1, space="PSUM") as ps_av,
            tc.tile_pool(name="ps_qkv", bufs=1, space="PSUM") as ps_qkv,
        ):
            # --- constants -------------------------------------------------
            wb_sb = constp.tile([1, OC], BF16)
            nc.sync.dma_start(out=wb_sb[:], in_=wb[:])
            wbp_sb = constp.tile([128, 12], F32)
            nc.sync.dma_start(out=wbp_sb[:], in_=wbp[:])
            wbv_b = constp.tile([128, HPC, HD], BF16)
            nc.gpsimd.partition_broadcast(
                wbv_b[:].rearrange("p h d -> p (h d)"),
                wb_sb[:, 2 * HPC * HD : 3 * HPC * HD],
            )
            # warm the ACT exp table before the first real exp
            junk = constp.tile([1, 8], F32)
            nc.vector.memset(junk[:], 0.0)
            nc.scalar.activation(junk[:], junk[:], mybir.ActivationFunctionType.Exp)

            # --- stage inputs ---------------------------------------------
            hT_sb = []
            wT_sb = []
            for c in range(DC):
                hwt = wp.tile([128, S + OC], BF16, tag=f"hw{c}", name=f"hw{c}")
                if c == 0:
                    nc.sync.dma_start(
                        out=hwt[0:64, :], in_=hw[0:64, :]
                    )
                    nc.sync.dma_start(
                        out=hwt[64:128, :], in_=hw[64:128, :]
                    )
                else:
                    nc.sync.dma_start(
                        out=hwt[:], in_=hw[c * 128 : (c + 1) * 128, :]
                    )
                hT_sb.append(hwt[:, 0:S])
                wT_sb.append(hwt[:, S : S + OC])

            # --- phase 1: fused QKV projection (emitted via filler queue) --
            # qk_sb[j][p, t]: j in 0..3 -> q rows (pre-scaled by 1/8),
            #                 j in 4..7 -> k rows. Row (j%4)*128+p = oc index.
            qk_sb = [
                qkp.tile([128, S], BF16, tag=f"qk{j}", name=f"qk{j}")
                for j in range(8)
            ]
            # v_sb[t][p, h, 0:64] = v head h, token t*128+p; [.., 64] = 1.0
            v_sb = [
                vp.tile([128, HPC, HD + 1], BF16, tag=f"vx{t}", name=f"v{t}")
                for t in range(KC)
            ]
            qk_ps: dict[tuple, object] = {}
            v_ps: dict[int, object] = {}

            def qk_mm(j, c):
                # both q-halves of block j share one LDWEIGHTS per chunk
                if c == 0:
                    qk_ps[(j, 0)] = ps_qkv.tile(
                        [128, 512], F32, tag="pA", name=f"qkpA{j}"
                    )[:]
                    qk_ps[(j, 1)] = ps_qkv.tile(
                        [128, 512], F32, tag="pB", name=f"qkpB{j}"
                    )[:]
                for half in range(2):
                    nc.tensor.matmul(
                        qk_ps[(j, half)],
                        wT_sb[c][:, j * 128 : (j + 1) * 128],
                        hT_sb[c][:, half * 512 : (half + 1) * 512],
                        start=(c == 0),
                        stop=(c == DC - 1),
                    )

            def qk_fin(j, half):
                ps = qk_ps.pop((j, half))
                dst = qk_sb[j][:, half * 512 : (half + 1) * 512]
                if j < 4:
                    nc.vector.tensor_scalar(
                        dst, ps, wbp_sb[:, j : j + 1], 0.125,
                        op0=mybir.AluOpType.add, op1=mybir.AluOpType.mult,
                    )
                else:
                    nc.vector.tensor_scalar_add(dst, ps, wbp_sb[:, j : j + 1])

            def v_mm(t, c):
                if c == 0:
                    v_ps[t] = ps_qkv.tile([128, 512], F32, tag="pA", name=f"vps{t}")
                nc.tensor.matmul(
                    v_ps[t][:],
                    hT_sb[c][:, t * 128 : (t + 1) * 128],
                    wT_sb[c][:, 2 * HPC * HD : 3 * HPC * HD],
                    start=(c == 0),
                    stop=(c == DC - 1),
                )

            def v_fin(t):
                ps = v_ps.pop(t)
                if not isinstance(ps, bass.AP):
                    ps = ps[:]
                nc.vector.tensor_tensor(
                    v_sb[t][:, :, 0:HD],
                    ps.rearrange("p (h d) -> p h d", h=HPC),
                    wbv_b[:],
                    op=mybir.AluOpType.add,
                )
                nc.scalar.activation(
                    v_sb[t][:, :, HD : HD + 1],
                    v_sb[t][:, :, 0:1],
                    mybir.ActivationFunctionType.Identity,
                    scale=0.0,
                    bias=1.0,
                )

            # Filler queue: (label-or-None, fn, est PE ns).
            SUB: list = []

            def add_qk_block(j):
                for c in range(DC):
                    SUB.append((None, (lambda j=j, c=c: qk_mm(j, c)), 460))
                SUB.append((None, (lambda j=j: qk_fin(j, 0)), 0))
                SUB.append((f"qk{j}", (lambda j=j: qk_fin(j, 1)), 0))

            def add_v_block(t):
                for c in range(DC):
                    SUB.append((None, (lambda t=t, c=c: v_mm(t, c)), 240))
                SUB.append((f"v{t}", (lambda t=t: v_fin(t)), 0))

            for blk in ["qk1", "qk5", "v2", "v3",
                        "v4", "qk2", "qk6", "v5", "v6", "v7", "qk3", "qk7"]:
                if blk.startswith("qk"):
                    add_qk_block(int(blk[2:]))
                else:
                    add_v_block(int(blk[1:]))

            total_sub_cost = sum(c for _, _, c in SUB)
            done_labels: set = set()
            sub_pos = [0]
            sub_spent = [0]

            def emit_next_sub():
                label, fn, cost = SUB[sub_pos[0]]
                fn()
                if label is not None:
                    done_labels.add(label)
                sub_pos[0] += 1
                sub_spent[0] += cost
                return cost

            def force(label):
                while label not in done_labels:
                    emit_next_sub()

            def filler_budget(ns):
                spent = 0
                while spent < ns and sub_pos[0] < len(SUB):
                    spent += emit_next_sub()

            # --- phase 2: attention ----------------------------------------
            # Per pair, A-items (kc, half) in this order; B-items follow one
            # pair behind in the same order.
            ORDER = (
                [(k, 0) for k in range(4)]
                + [(k, 0) for k in range(4, 8)]
                + [(k, 1) for k in range(4)]
                + [(k, 1) for k in range(4, 8)]
            )
            ebs: dict[tuple, object] = {}
            ets: dict[tuple, object] = {}
            av_ps: dict[int, object] = {}
            dn_ps: dict[int, object] = {}

            def eb_fetch(pair, kc, half):
                t = ebp.tile([128, S], BF16, tag="eb", name=f"eb{pair}_{kc}_{half}")
                nc.sync.dma_start(out=t[:], in_=eb[pair, kc, half])
                ebs[(pair, kc, half)] = t

            def a_item(pair, kc, half):
                qT = qk_sb[pair]
                kT = qk_sb[4 + pair]
                sc = ps_sc.tile([128, S], F32, tag="sc", name=f"s{pair}_{kc}_{half}")
                for hh in range(2):  # head hh of the pair -> cols hh*512
                    p0 = hh * 64
                    nc.tensor.matmul(
                        sc[:, hh * 512 : (hh + 1) * 512],
                        kT[p0 : p0 + 64, kc * 128 : (kc + 1) * 128],
                        qT[p0 : p0 + 64, half * 512 : (half + 1) * 512],
                        start=True,
                        stop=True,
                    )
                et = ep.tile([128, S], BF16, tag="et", name=f"et{pair}_{kc}_{half}")
                nc.scalar.activation(et[:], sc[:], mybir.ActivationFunctionType.Exp)
                ebt = ebs.pop((pair, kc, half))
                nc.vector.tensor_tensor(et[:], et[:], ebt[:], op=mybir.AluOpType.mult)
                ets[(pair, kc, half)] = et

            def b_item(pair, kc, half, first):
                force(f"v{kc}")
                et = ets.pop((pair, kc, half))
                for hh in range(2):
                    h = 2 * pair + hh
                    if kc == 0:
                        av_ps[(pair, hh, half)] = ps_av.tile(
                            [HD + 1, 512], F32, tag=f"av{hh}{half}",
                            name=f"av{pair}_{hh}_{half}",
                        )
                    nc.tensor.matmul(
                        av_ps[(pair, hh, half)][:],
                        v_sb[kc][:, h, :],
                        et[:, hh * 512 : (hh + 1) * 512],
                        start=(kc == 0),
                        stop=(kc == KC - 1),
                    )

            def emit_tail_half(pair, half, last):
                cs = slice(half * 512, (half + 1) * 512)
                for hh in range(2):
                    h = 2 * pair + hh
                    av = av_ps.pop((pair, hh, half))
                    smf = op_.tile([1, 512], F32, tag="smf")
                    nc.scalar.activation(
                        smf[:], av[HD : HD + 1, :],
                        mybir.ActivationFunctionType.Copy,
                    )
                    rcf = op_.tile([1, 512], F32, tag="rcf")
                    nc.vector.reciprocal_approx_fast(rcf[:], smf[:])
                    rb = op_.tile([HD, 512], F32, tag="rb")
                    nc.gpsimd.partition_broadcast(rb[:], rcf[:])
                    ot = op_.tile([HD, 512], BF16, tag="ot")
                    nc.vector.tensor_tensor(
                        ot[:], av[0:HD, :], rb[:],
                        op=mybir.AluOpType.mult,
                    )
                    nc.sync.dma_start(
                        out=oT[h * HD : (h + 1) * HD, cs], in_=ot[:]
                    )

            b_queue: list = []

            def b_step():
                if not b_queue:
                    return
                pair, (kc, half), first, last = b_queue.pop(0)
                b_item(pair, kc, half, first)
                if kc == KC - 1:
                    emit_tail_half(pair, half, last)

            # --- prelude: qk0 + qk4 chunk-major so each arriving hw chunk
            # feeds 4 back-to-back matmuls (keeps the PE dense + HAM warm).
            # qk0 borrows the (idle) AV PSUM banks; qk4 uses the qkv pool.
            qk_ps[(0, 0)] = ps_av.tile(
                [128, 512], F32, tag="av00", name="pre_qk0a"
            )[:]
            qk_ps[(0, 1)] = ps_av.tile(
                [128, 512], F32, tag="av10", name="pre_qk0b"
            )[:]
            qk_ps[(4, 0)] = ps_qkv.tile([128, 512], F32, tag="pA", name="pre_qk4a")
            qk_ps[(4, 1)] = ps_qkv.tile([128, 512], F32, tag="pB", name="pre_qk4b")
            pre_v = ps_sc.tile([128, S], F32, tag="sc", name="pre_v01")
            v_ps[0] = pre_v[:, 0:512]
            v_ps[1] = pre_v[:, 512:1024]
            for c in range(DC):
                for j in (0, 4):
                    for half in range(2):
                        nc.tensor.matmul(
                            qk_ps[(j, half)],
                            wT_sb[c][:, j * 128 : (j + 1) * 128],
                            hT_sb[c][:, half * 512 : (half + 1) * 512],
                            start=(c == 0),
                            stop=(c == DC - 1),
                        )
                for t in (0, 1):
                    nc.tensor.matmul(
                        v_ps[t],
                        hT_sb[c][:, t * 128 : (t + 1) * 128],
                        wT_sb[c][:, 2 * HPC * HD : 3 * HPC * HD],
                        start=(c == 0),
                        stop=(c == DC - 1),
                    )
            for j in (0, 4):
                for half in range(2):
                    qk_fin(j, half)
            for t in (0, 1):
                v_fin(t)
            done_labels.add("qk0")
            done_labels.add("qk4")
            done_labels.add("v0")
            done_labels.add("v1")

            NI = NPAIR * len(ORDER)  # 64
            n_done = 0
            for pair in range(NPAIR):
                for oi, (kc, half) in enumerate(ORDER):
                    if oi == 0:
                        force(f"qk{pair}")
                        force(f"qk{4 + pair}")
                        for pf in range(EB_PREFETCH):
                            eb_fetch(pair, *ORDER[pf])
                    if oi + EB_PREFETCH < 16:
                        eb_fetch(pair, *ORDER[oi + EB_PREFETCH])
                    a_item(pair, kc, half)
                    b_queue.append(
                        (pair, (kc, half), oi == 0, oi == len(ORDER) - 1)
                    )
                    # consume b-items with a short lag (covers exp+mult
                    # latency); per-(head,half) av tiles make bank handoff
                    # between pairs dependency-free
                    steps = 0
                    while len(b_queue) > 8 and steps < 2:
                        b_step()
                        steps += 1
                    n_done += 1
                    # adaptive filler: spread remaining qkv over remaining items
                    rem_items = NI - n_done
                    rem_cost = total_sub_cost - sub_spent[0]
                    if rem_items > 0 and rem_cost > 0:
                        filler_budget(min(1500, rem_cost // rem_items + 1))
            while b_queue:
                b_step()
            while sub_pos[0] < len(SUB):
                emit_next_sub()

    nc.finalize()
    return nc


def shard_inputs(hidden_states, bias, Wqkv_w, Wqkv_b):
    """Slice + lay out the full inputs into 8 per-core input maps."""
    import ml_dtypes

    bf16 = ml_dtypes.bfloat16
    hidden_states = np.asarray(hidden_states, dtype=np.float32)
    bias = np.asarray(bias, dtype=np.float32)
    Wqkv_w = np.asarray(Wqkv_w, dtype=np.float32)
    Wqkv_b = np.asarray(Wqkv_b, dtype=np.float32)

    expb = np.exp(bias)  # (B, H, S, S) fp32

    in_maps = []
    for c in range(N_CORES):
        b, hs = c // 2, (c % 2) * HPC
        rows = np.concatenate(
            [np.arange(g * D + hs * HD, g * D + (hs + HPC) * HD) for g in range(3)]
        )
        wb2 = Wqkv_b[rows][None, :].astype(bf16)
        wbp2 = np.ascontiguousarray(
            Wqkv_b[rows].reshape(12, 128).T
        ).astype(np.float32)
        # eb[pair, kc, half, 128, 1024]: cols 0:512 = head hs+2*pair,
        # cols 512:1024 = head hs+2*pair+1; [k in chunk kc] x [q in half].
        ebt = np.empty((NPAIR, KC, 2, 128, S), dtype=bf16)
        for pair in range(NPAIR):
            for hh in range(2):
                h = hs + 2 * pair + hh
                e = expb[b, h].T.astype(bf16)  # [k, q]
                er = e.reshape(KC, 128, 2, 512)
                for kc in range(KC):
                    for half in range(2):
                        ebt[pair, kc, half, :, hh * 512 : (hh + 1) * 512] = er[
                            kc, :, half
                        ]
        in_maps.append(
            {
                "hw": np.concatenate(
                    [hidden_states[b].T, Wqkv_w[rows].T], axis=1
                ).astype(bf16),
                "wb": wb2,
                "wbp": wbp2,
                "eb": ebt,
            }
        )
    return in_maps


_CACHED_NC = None


def kernel(hidden_states, bias, Wqkv_w, Wqkv_b):
    from concourse.bass_utils import run_bass_kernel_spmd

    global _CACHED_NC
    if _CACHED_NC is None:
        _CACHED_NC = build_bass()
    in_maps = shard_inputs(hidden_states, bias, Wqkv_w, Wqkv_b)
    res = run_bass_kernel_spmd(_CACHED_NC, in_maps, core_ids=list(range(N_CORES)))
    out = np.empty((B, S, D), dtype=np.float32)
    for c in range(N_CORES):
        b, hs = c // 2, (c % 2) * HPC
        out[b, :, hs * HD : (hs + HPC) * HD] = res.results[c]["oT"].T
    return out


# revision 15
# speedup vs baseline: 1.1744x; 1.1744x over previous
"""BertSelfAttention (ALiBi-style additive bias) on 8 TRN2 NeuronCores.

Problem: B=4, S=1024, D=1024, H=16 heads (HD=64), fp32.
  qkv = hidden @ Wqkv_w.T + Wqkv_b
  scores = q @ k.T / sqrt(64) + bias ;  probs = softmax(scores) ; out = probs @ v

Sharding: 8 cores = 4 batches x 2 head-groups. Core c handles batch c//2 and
heads [ (c%2)*8, (c%2)*8+8 ).  Per-core shards are prepared host-side in the
layouts the TensorEngine wants (contraction dim on partitions) and cast to
bf16, so every device DMA is a contiguous, full-rate read:
  hw  [D, S+1536]  = [hidden[b].T | Wqkv rows for this core, transposed]
  eb  [4,8,2,128,1024] = exp(bias^T) tiles packed per (head-pair, kc, q-half)
                     with the two heads of a pair side by side in columns

Device algorithm (per core), exploiting exp(s+b) = exp(s)*exp(b):
  - QKV projection as in the baseline (bf16 matmuls, fp32 PSUM), q rows
    pre-scaled by 1/8; the q/k rows of a head pair land on partitions
    0:64 / 64:128 of the qk tiles.
  - scores: per (pair, kc, q-half), TWO row-tiled K=64 matmuls run
    concurrently on the PE array (tile_position (0,0) and (64,0)), writing
    head0 -> cols 0:512, head1 -> cols 512:1024 of one [128,1024] PSUM tile.
  - exp on ScalarE (PSUM->SBUF bf16), then DVE multiply by the DMA'd
    exp(bias) tile (this replaces the baseline's identity-matmul bias add).
  - AV: per item, TWO col-tiled M=64 matmuls (tile_position (0,0)/(0,64))
    accumulate both heads' outT into one [128,1024] PSUM tile; TWO col-tiled
    M=1 ones-matmuls accumulate the softmax denominators.
  - normalize: reciprocal on DVE, partition-broadcast on GpSimd, bf16
    multiply on DVE, DMA out (host re-transposes).
QKV block emission is interleaved into the attention item loop (the filler
queue) so the in-order PE stream keeps the ScalarE exp pipeline fed.
PSUM budget (8 banks): scores 2 + AV 2 + denom 2 + qkv-proj 1 + v-proj 1.
"""

import numpy as np

import concourse.bacc as bacc
import concourse.bass as bass
import concourse.mybir as mybir
from concourse.tile import TileContext

B, S, D = 4, 1024, 1024
H = 16
HD = 64  # head dim
N_CORES = 8
HPC = 8  # heads per core
NPAIR = HPC // 2
OC = 3 * HPC * HD  # 1536 fused-qkv output rows per core
F32 = mybir.dt.float32
BF16 = mybir.dt.bfloat16

KC = S // 128  # 8 key-token chunks of 128
DC = D // 128  # 8 contraction chunks of 128
EB_PREFETCH = 11


def build_bass() -> bass.Bass:
    nc = bacc.Bacc()

    hw = nc.declare_dram_parameter("hw", [D, S + OC], BF16, isOutput=False)
    wb = nc.declare_dram_parameter("wb", [1, OC], BF16, isOutput=False)
    wbp = nc.declare_dram_parameter("wbp", [128, 12], F32, isOutput=False)
    eb = nc.declare_dram_parameter("eb", [NPAIR, KC, 2, 128, S], BF16, isOutput=False)
    oT = nc.declare_dram_parameter("oT", [HPC * HD, S], BF16, isOutput=True)

    with TileContext(nc) as tc:
        with (
            tc.tile_pool(name="const", bufs=1) as constp,
            tc.tile_pool(name="weights", bufs=1) as wp,
            tc.tile_pool(name="qk", bufs=1) as qkp,
            tc.tile_pool(name="vex", bufs=1) as vp,
            tc.tile_pool(name="ebias", bufs=16) as ebp,
            tc.tile_pool(name="exp", bufs=14) as ep,
            tc.tile_pool(name="outs", bufs=4) as op_,
            tc.tile_pool(name="ps_sc", bufs=1, space="PSUM") as ps_sc,
            tc.tile_pool(name="ps_av", bufs=1, space="PSUM") as ps_av,
            tc.tile_pool(name="ps_qkv", bufs=1, space="PSUM") as ps_qkv,
        ):
            # --- constants -------------------------------------------------
            wb_sb = constp.tile([1, OC], BF16)
            nc.sync.dma_start(out=wb_sb[:], in_=wb[:])
            wbp_sb = constp.tile([128, 12], F32)
            nc.sync.dma_start(out=wbp_sb[:], in_=wbp[:])
            wbv_b = constp.tile([128, HPC, HD], BF16)
            nc.gpsimd.partition_broadcast(
                wbv_b[:].rearrange("p h d -> p (h d)"),
                wb_sb[:, 2 * HPC * HD : 3 * HPC * HD],
            )
            # warm the ACT exp table before the first real exp
            junk = constp.tile([1, 8], F32)
            nc.vector.memset(junk[:], 0.0)
            nc.scalar.activation(junk[:], junk[:], mybir.ActivationFunctionType.Exp)

            # --- stage inputs ---------------------------------------------
            hT_sb = []
            wT_sb = []
            for c in range(DC):
                hwt = wp.tile([128, S + OC], BF16, tag=f"hw{c}", name=f"hw{c}")
                if c == 0:
                    nc.sync.dma_start(
                        out=hwt[0:64, :], in_=hw[0:64, :]
                    )
                    nc.sync.dma_start(
                        out=hwt[64:128, :], in_=hw[64:128, :]
                    )
                else:
                    nc.sync.dma_start(
                        out=hwt[:], in_=hw[c * 128 : (c + 1) * 128, :]
                    )
                hT_sb.append(hwt[:, 0:S])
                wT_sb.append(hwt[:, S : S + OC])

            # --- phase 1: fused QKV projection (emitted via filler queue) --
            # qk_sb[j][p, t]: j in 0..3 -> q rows (pre-scaled by 1/8),
            #                 j in 4..7 -> k rows. Row (j%4)*128+p = oc index.
            qk_sb = [
                qkp.tile([128, S], BF16, tag=f"qk{j}", name=f"qk{j}")
                for j in range(8)
            ]
            # v_sb[t][p, h, 0:64] = v head h, token t*128+p; [.., 64] = 1.0
            # (the constant ones column is written up-front so the AV matmuls
            # never wait on the ScalarE queue for it)
            v_sb = []
            for t in range(KC):
                vt = vp.tile([128, HPC, HD + 1], BF16, tag=f"vx{t}", name=f"v{t}")
                nc.vector.memset(vt[:, :, HD : HD + 1], 1.0)
                v_sb.append(vt)
            qk_ps: dict[tuple, object] = {}
            v_ps: dict[int, object] = {}

            def qk_mm(j, c):
                # both q-halves of block j share one LDWEIGHTS per chunk
                if c == 0:
                    qk_ps[(j, 0)] = ps_qkv.tile(
                        [128, 512], F32, tag="pA", name=f"qkpA{j}"
                    )[:]
                    qk_ps[(j, 1)] = ps_qkv.tile(
                        [128, 512], F32, tag="pB", name=f"qkpB{j}"
                    )[:]
                for half in range(2):
                    nc.tensor.matmul(
                        qk_ps[(j, half)],
                        wT_sb[c][:, j * 128 : (j + 1) * 128],
                        hT_sb[c][:, half * 512 : (half + 1) * 512],
                        start=(c == 0),
                        stop=(c == DC - 1),
                    )

            def qk_fin(j, half):
                ps = qk_ps.pop((j, half))
                dst = qk_sb[j][:, half * 512 : (half + 1) * 512]
                if j < 4:
                    nc.vector.tensor_scalar(
                        dst, ps, wbp_sb[:, j : j + 1], 0.125,
                        op0=mybir.AluOpType.add, op1=mybir.AluOpType.mult,
                    )
                else:
                    nc.vector.tensor_scalar_add(dst, ps, wbp_sb[:, j : j + 1])

            def v_mm(t, c):
                if c == 0:
                    v_ps[t] = ps_qkv.tile([128, 512], F32, tag="pA", name=f"vps{t}")
                nc.tensor.matmul(
                    v_ps[t][:],
                    hT_sb[c][:, t * 128 : (t + 1) * 128],
                    wT_sb[c][:, 2 * HPC * HD : 3 * HPC * HD],
                    start=(c == 0),
                    stop=(c == DC - 1),
                )

            def v_fin(t):
                ps = v_ps.pop(t)
                if not isinstance(ps, bass.AP):
                    ps = ps[:]
                nc.vector.tensor_tensor(
                    v_sb[t][:, :, 0:HD],
                    ps.rearrange("p (h d) -> p h d", h=HPC),
                    wbv_b[:],
                    op=mybir.AluOpType.add,
                )


            # Filler queue: (label-or-None, fn, est PE ns).
            SUB: list = []

            def add_qk_block(j):
                for c in range(DC):
                    SUB.append((None, (lambda j=j, c=c: qk_mm(j, c)), 460))
                SUB.append((None, (lambda j=j: qk_fin(j, 0)), 0))
                SUB.append((f"qk{j}", (lambda j=j: qk_fin(j, 1)), 0))

            def add_v_block(t):
                for c in range(DC):
                    SUB.append((None, (lambda t=t, c=c: v_mm(t, c)), 240))
                SUB.append((f"v{t}", (lambda t=t: v_fin(t)), 0))

            for blk in ["qk1", "qk5", "v2", "v3",
                        "v4", "qk2", "qk6", "v5", "v6", "v7", "qk3", "qk7"]:
                if blk.startswith("qk"):
                    add_qk_block(int(blk[2:]))
                else:
                    add_v_block(int(blk[1:]))

            total_sub_cost = sum(c for _, _, c in SUB)
            done_labels: set = set()
            sub_pos = [0]
            sub_spent = [0]

            def emit_next_sub():
                label, fn, cost = SUB[sub_pos[0]]
                fn()
                if label is not None:
                    done_labels.add(label)
                sub_pos[0] += 1
                sub_spent[0] += cost
                return cost

            def force(label):
                while label not in done_labels:
                    emit_next_sub()

            def filler_budget(ns):
                spent = 0
                while spent < ns and sub_pos[0] < len(SUB):
                    spent += emit_next_sub()

            # --- phase 2: attention ----------------------------------------
            # Per pair, A-items (kc, half) in this order; B-items follow one
            # pair behind in the same order.
            ORDER = (
                [(k, 0) for k in range(4)]
                + [(k, 0) for k in range(4, 8)]
                + [(k, 1) for k in range(4)]
                + [(k, 1) for k in range(4, 8)]
            )
            ebs: dict[tuple, object] = {}
            ets: dict[tuple, object] = {}
            av_ps: dict[int, object] = {}
            dn_ps: dict[int, object] = {}

            def eb_fetch(pair, kc, half):
                t = ebp.tile([128, S], BF16, tag="eb", name=f"eb{pair}_{kc}_{half}")
                nc.sync.dma_start(out=t[:], in_=eb[pair, kc, half])
                ebs[(pair, kc, half)] = t

            def a_item(pair, kc, half):
                qT = qk_sb[pair]
                kT = qk_sb[4 + pair]
                sc = ps_sc.tile([128, S], F32, tag="sc", name=f"s{pair}_{kc}_{half}")
                for hh in range(2):  # head hh of the pair -> cols hh*512
                    p0 = hh * 64
                    nc.tensor.matmul(
                        sc[:, hh * 512 : (hh + 1) * 512],
                        kT[p0 : p0 + 64, kc * 128 : (kc + 1) * 128],
                        qT[p0 : p0 + 64, half * 512 : (half + 1) * 512],
                        start=True,
                        stop=True,
                    )
                et = ep.tile([128, S], BF16, tag="et", name=f"et{pair}_{kc}_{half}")
                nc.scalar.activation(et[:], sc[:], mybir.ActivationFunctionType.Exp)
                ebt = ebs.pop((pair, kc, half))
                nc.vector.tensor_tensor(et[:], et[:], ebt[:], op=mybir.AluOpType.mult)
                ets[(pair, kc, half)] = et

            def b_item(pair, kc, half, first):
                force(f"v{kc}")
                et = ets.pop((pair, kc, half))
                for hh in range(2):
                    h = 2 * pair + hh
                    if kc == 0:
                        av_ps[(pair, hh, half)] = ps_av.tile(
                            [HD + 1, 512], F32, tag=f"av{hh}{half}",
                            name=f"av{pair}_{hh}_{half}",
                        )
                    nc.tensor.matmul(
                        av_ps[(pair, hh, half)][:],
                        v_sb[kc][:, h, :],
                        et[:, hh * 512 : (hh + 1) * 512],
                        start=(kc == 0),
                        stop=(kc == KC - 1),
                    )

            def emit_tail_half(pair, half, last):
                cs = slice(half * 512, (half + 1) * 512)
                for hh in range(2):
                    h = 2 * pair + hh
                    av = av_ps.pop((pair, hh, half))
                    smf = op_.tile([1, 512], F32, tag="smf")
                    nc.scalar.activation(
                        smf[:], av[HD : HD + 1, :],
                        mybir.ActivationFunctionType.Copy,
                    )
                    rcf = op_.tile([1, 512], F32, tag="rcf")
                    nc.vector.reciprocal_approx_fast(rcf[:], smf[:])
                    rb = op_.tile([HD, 512], F32, tag="rb")
                    nc.gpsimd.partition_broadcast(rb[:], rcf[:])
                    ot = op_.tile([HD, 512], BF16, tag="ot")
                    nc.vector.tensor_tensor(
                        ot[:], av[0:HD, :], rb[:],
                        op=mybir.AluOpType.mult,
                    )
                    nc.sync.dma_start(
                        out=oT[h * HD : (h + 1) * HD, cs], in_=ot[:]
                    )

            b_queue: list = []

            def b_step():
                if not b_queue:
                    return
                pair, (kc, half), first, last = b_queue.pop(0)
                b_item(pair, kc, half, first)
                if kc == KC - 1:
                    emit_tail_half(pair, half, last)

            # --- prelude: qk0 + qk4 chunk-major so each arriving hw chunk
            # feeds 4 back-to-back matmuls (keeps the PE dense + HAM warm).
            # qk0 borrows the (idle) AV PSUM banks; qk4 uses the qkv pool.
            qk_ps[(0, 0)] = ps_av.tile(
                [128, 512], F32, tag="av00", name="pre_qk0a"
            )[:]
            qk_ps[(0, 1)] = ps_av.tile(
                [128, 512], F32, tag="av10", name="pre_qk0b"
            )[:]
            qk_ps[(4, 0)] = ps_qkv.tile([128, 512], F32, tag="pA", name="pre_qk4a")
            qk_ps[(4, 1)] = ps_qkv.tile([128, 512], F32, tag="pB", name="pre_qk4b")
            pre_v = ps_sc.tile([128, S], F32, tag="sc", name="pre_v01")
            v_ps[0] = pre_v[:, 0:512]
            v_ps[1] = pre_v[:, 512:1024]
            for c in range(DC):
                for j in (0, 4):
                    for half in range(2):
                        nc.tensor.matmul(
                            qk_ps[(j, half)],
                            wT_sb[c][:, j * 128 : (j + 1) * 128],
                            hT_sb[c][:, half * 512 : (half + 1) * 512],
                            start=(c == 0),
                            stop=(c == DC - 1),
                        )
                for t in (0, 1):
                    nc.tensor.matmul(
                        v_ps[t],
                        hT_sb[c][:, t * 128 : (t + 1) * 128],
                        wT_sb[c][:, 2 * HPC * HD : 3 * HPC * HD],
                        start=(c == 0),
                        stop=(c == DC - 1),
                    )
            for j in (0, 4):
                for half in range(2):
                    qk_fin(j, half)
            for t in (0, 1):
                v_fin(t)
            done_labels.add("qk0")
            done_labels.add("qk4")
            done_labels.add("v0")
            done_labels.add("v1")

            NI = NPAIR * len(ORDER)  # 64
            n_done = 0
            for pair in range(NPAIR):
                for oi, (kc, half) in enumerate(ORDER):
                    if oi == 0:
                        force(f"qk{pair}")
                        force(f"qk{4 + pair}")
                        for pf in range(EB_PREFETCH):
                            eb_fetch(pair, *ORDER[pf])
                    if oi + EB_PREFETCH < 16:
                        eb_fetch(pair, *ORDER[oi + EB_PREFETCH])
                    a_item(pair, kc, half)
                    b_queue.append(
                        (pair, (kc, half), oi == 0, oi == len(ORDER) - 1)
                    )
                    # consume b-items with a short lag (covers exp+mult
                    # latency); per-(head,half) av tiles make bank handoff
                    # between pairs dependency-free
                    steps = 0
                    while len(b_queue) > 8 and steps < 2:
                        b_step()
                        steps += 1
                    n_done += 1
                    # adaptive filler: spread remaining qkv over remaining items
                    rem_items = NI - n_done
                    rem_cost = total_sub_cost - sub_spent[0]
                    if rem_items > 0 and rem_cost > 0:
                        filler_budget(min(1500, rem_cost // rem_items + 1))
            while b_queue:
                b_step()
            while sub_pos[0] < len(SUB):
                emit_next_sub()

    nc.finalize()
    return nc


def shard_inputs(hidden_states, bias, Wqkv_w, Wqkv_b):
    """Slice + lay out the full inputs into 8 per-core input maps."""
    import ml_dtypes

    bf16 = ml_dtypes.bfloat16
    hidden_states = np.asarray(hidden_states, dtype=np.float32)
    bias = np.asarray(bias, dtype=np.float32)
    Wqkv_w = np.asarray(Wqkv_w, dtype=np.float32)
    Wqkv_b = np.asarray(Wqkv_b, dtype=np.float32)

    expb = np.exp(bias)  # (B, H, S, S) fp32

    in_maps = []
    for c in range(N_CORES):
        b, hs = c // 2, (c % 2) * HPC
        rows = np.concatenate(
            [np.arange(g * D + hs * HD, g * D + (hs + HPC) * HD) for g in range(3)]
        )
        wb2 = Wqkv_b[rows][None, :].astype(bf16)
        wbp2 = np.ascontiguousarray(
            Wqkv_b[rows].reshape(12, 128).T
        ).astype(np.float32)
        # eb[pair, kc, half, 128, 1024]: cols 0:512 = head hs+2*pair,
        # cols 512:1024 = head hs+2*pair+1; [k in chunk kc] x [q in half].
        ebt = np.empty((NPAIR, KC, 2, 128, S), dtype=bf16)
        for pair in range(NPAIR):
            for hh in range(2):
                h = hs + 2 * pair + hh
                e = expb[b, h].T.astype(bf16)  # [k, q]
                er = e.reshape(KC, 128, 2, 512)
                for kc in range(KC):
                    for half in range(2):
                        ebt[pair, kc, half, :, hh * 512 : (hh + 1) * 512] = er[
                            kc, :, half
                        ]
        in_maps.append(
            {
                "hw": np.concatenate(
                    [hidden_states[b].T, Wqkv_w[rows].T], axis=1
                ).astype(bf16),
                "wb": wb2,
                "wbp": wbp2,
                "eb": ebt,
            }
        )
    return in_maps


_CACHED_NC = None


def kernel(hidden_states, bias, Wqkv_w, Wqkv_b):
    from concourse.bass_utils import run_bass_kernel_spmd

    global _CACHED_NC
    if _CACHED_NC is None:
        _CACHED_NC = build_bass()
    in_maps = shard_inputs(hidden_states, bias, Wqkv_w, Wqkv_b)
    res = run_bass_kernel_spmd(_CACHED_NC, in_maps, core_ids=list(range(N_CORES)))
    out = np.empty((B, S, D), dtype=np.float32)
    for c in range(N_CORES):
        b, hs = c // 2, (c % 2) * HPC
        out[b, :, hs * HD : (hs + HPC) * HD] = res.results[c]["oT"].T
    return out


# revision 16
# speedup vs baseline: 1.3173x; 1.1217x over previous
"""BertSelfAttention (ALiBi-style additive bias) on 8 TRN2 NeuronCores.

Problem: B=4, S=1024, D=1024, H=16 heads (HD=64), fp32.
  qkv = hidden @ Wqkv_w.T + Wqkv_b
  scores = q @ k.T / sqrt(64) + bias ;  probs = softmax(scores) ; out = probs @ v

Sharding: 8 cores = 4 batches x 2 head-groups. Core c handles batch c//2 and
heads [ (c%2)*8, (c%2)*8+8 ).  Per-core shards are prepared host-side in the
layouts the TensorEngine wants (contraction dim on partitions) and cast to
bf16, so every device DMA is a contiguous, full-rate read:
  hw  [D, S+1536]  = [hidden[b].T | Wqkv rows for this core, transposed]
  eb  [4,8,2,128,1024] = exp(bias^T) tiles packed per (head-pair, kc, q-half)
                     with the two heads of a pair side by side in columns

Device algorithm (per core), exploiting exp(s+b) = exp(s)*exp(b):
  - QKV projection as in the baseline (bf16 matmuls, fp32 PSUM), q rows
    pre-scaled by 1/8; the q/k rows of a head pair land on partitions
    0:64 / 64:128 of the qk tiles.
  - scores: per (pair, kc, q-half), TWO row-tiled K=64 matmuls run
    concurrently on the PE array (tile_position (0,0) and (64,0)), writing
    head0 -> cols 0:512, head1 -> cols 512:1024 of one [128,1024] PSUM tile.
  - exp on ScalarE (PSUM->SBUF bf16), then DVE multiply by the DMA'd
    exp(bias) tile (this replaces the baseline's identity-matmul bias add).
  - AV: per item, TWO col-tiled M=64 matmuls (tile_position (0,0)/(0,64))
    accumulate both heads' outT into one [128,1024] PSUM tile; TWO col-tiled
    M=1 ones-matmuls accumulate the softmax denominators.
  - normalize: reciprocal on DVE, partition-broadcast on GpSimd, bf16
    multiply on DVE, DMA out (host re-transposes).
QKV block emission is interleaved into the attention item loop (the filler
queue) so the in-order PE stream keeps the ScalarE exp pipeline fed.
PSUM budget (8 banks): scores 2 + AV 2 + denom 2 + qkv-proj 1 + v-proj 1.
"""

import numpy as np

import concourse.bacc as bacc
import concourse.bass as bass
import concourse.mybir as mybir
from concourse.tile import TileContext

B, S, D = 4, 1024, 1024
H = 16
HD = 64  # head dim
N_CORES = 8
HPC = 8  # heads per core
NPAIR = HPC // 2
OC = 3 * HPC * HD  # 1536 fused-qkv output rows per core
F32 = mybir.dt.float32
BF16 = mybir.dt.bfloat16

KC = S // 128  # 8 key-token chunks of 128
DC = D // 128  # 8 contraction chunks of 128
EB_PREFETCH = 8


def build_bass() -> bass.Bass:
    nc = bacc.Bacc()

    hw = nc.declare_dram_parameter("hw", [D, S + OC], BF16, isOutput=False)
    wb = nc.declare_dram_parameter("wb", [1, OC], BF16, isOutput=False)
    wbp = nc.declare_dram_parameter("wbp", [128, 12], F32, isOutput=False)
    eb = nc.declare_dram_parameter("eb", [NPAIR, KC, 2, 128, S], BF16, isOutput=False)
    oT = nc.declare_dram_parameter("oT", [HPC * HD, S], BF16, isOutput=True)

    with TileContext(nc) as tc:
        with (
            tc.tile_pool(name="const", bufs=1) as constp,
            tc.tile_pool(name="weights", bufs=1) as wp,
            tc.tile_pool(name="qk", bufs=1) as qkp,
            tc.tile_pool(name="vex", bufs=1) as vp,
            tc.tile_pool(name="ebias", bufs=10) as ebp,
            tc.tile_pool(name="exp", bufs=20) as ep,
            tc.tile_pool(name="outs", bufs=4) as op_,
            tc.tile_pool(name="ps_sc", bufs=1, space="PSUM") as ps_sc,
            tc.tile_pool(name="ps_av", bufs=1, space="PSUM") as ps_av,
            tc.tile_pool(name="ps_dn", bufs=1, space="PSUM") as ps_dn,
            tc.tile_pool(name="ps_qkv", bufs=1, space="PSUM") as ps_qkv,
        ):
            # --- constants -------------------------------------------------
            wb_sb = constp.tile([1, OC], BF16)
            nc.sync.dma_start(out=wb_sb[:], in_=wb[:])
            wbp_sb = constp.tile([128, 12], F32)
            nc.sync.dma_start(out=wbp_sb[:], in_=wbp[:])
            wbv_b = constp.tile([128, HPC, HD], BF16)
            nc.gpsimd.partition_broadcast(
                wbv_b[:].rearrange("p h d -> p (h d)"),
                wb_sb[:, 2 * HPC * HD : 3 * HPC * HD],
            )
            ones_col = constp.tile([128, 1], BF16)
            nc.vector.memset(ones_col[:], 1.0)
            # warm the ACT exp table before the first real exp
            junk = constp.tile([1, 8], F32)
            nc.vector.memset(junk[:], 0.0)
            nc.scalar.activation(junk[:], junk[:], mybir.ActivationFunctionType.Exp)

            # --- stage inputs ---------------------------------------------
            hT_sb = []
            wT_sb = []
            for c in range(DC):
                hwt = wp.tile([128, S + OC], BF16, tag=f"hw{c}", name=f"hw{c}")
                nc.sync.dma_start(out=hwt[:], in_=hw[c * 128 : (c + 1) * 128, :])
                hT_sb.append(hwt[:, 0:S])
                wT_sb.append(hwt[:, S : S + OC])

            # --- phase 1: fused QKV projection (emitted via filler queue) --
            # qk_sb[j][p, t]: j in 0..3 -> q rows (pre-scaled by 1/8),
            #                 j in 4..7 -> k rows. Row (j%4)*128+p = oc index.
            qk_sb = [
                qkp.tile([128, S], BF16, tag=f"qk{j}", name=f"qk{j}")
                for j in range(8)
            ]
            # v_sb[t][p, h, :] = v head h, token t*128+p
            v_sb = [
                vp.tile([128, HPC, HD], BF16, tag=f"vx{t}", name=f"v{t}")
                for t in range(KC)
            ]
            qk_ps: dict[tuple, object] = {}
            v_ps: dict[int, object] = {}

            def qk_mm(j, c):
                # both q-halves of block j share one LDWEIGHTS per chunk
                if c == 0:
                    qk_ps[(j, 0)] = ps_qkv.tile(
                        [128, 512], F32, tag="pA", name=f"qkpA{j}"
                    )[:]
                    qk_ps[(j, 1)] = ps_qkv.tile(
                        [128, 512], F32, tag="pB", name=f"qkpB{j}"
                    )[:]
                for half in range(2):
                    nc.tensor.matmul(
                        qk_ps[(j, half)],
                        wT_sb[c][:, j * 128 : (j + 1) * 128],
                        hT_sb[c][:, half * 512 : (half + 1) * 512],
                        start=(c == 0),
                        stop=(c == DC - 1),
                    )

            def qk_fin(j, half):
                ps = qk_ps.pop((j, half))
                dst = qk_sb[j][:, half * 512 : (half + 1) * 512]
                if j < 4:
                    nc.vector.tensor_scalar(
                        dst, ps, wbp_sb[:, j : j + 1], 0.125,
                        op0=mybir.AluOpType.add, op1=mybir.AluOpType.mult,
                    )
                else:
                    nc.vector.tensor_scalar_add(dst, ps, wbp_sb[:, j : j + 1])

            def v_mm(t, c):
                if c == 0:
                    v_ps[t] = ps_qkv.tile([128, 512], F32, tag="pA", name=f"vps{t}")
                nc.tensor.matmul(
                    v_ps[t][:],
                    hT_sb[c][:, t * 128 : (t + 1) * 128],
                    wT_sb[c][:, 2 * HPC * HD : 3 * HPC * HD],
                    start=(c == 0),
                    stop=(c == DC - 1),
                )

            def v_fin(t):
                ps = v_ps.pop(t)
                if not isinstance(ps, bass.AP):
                    ps = ps[:]
                nc.vector.tensor_tensor(
                    v_sb[t][:],
                    ps.rearrange("p (h d) -> p h d", h=HPC),
                    wbv_b[:],
                    op=mybir.AluOpType.add,
                )


            # Filler queue: (label-or-None, fn, est PE ns).
            SUB: list = []

            def add_qk_block(j):
                for c in range(DC):
                    SUB.append((None, (lambda j=j, c=c: qk_mm(j, c)), 460))
                SUB.append((None, (lambda j=j: qk_fin(j, 0)), 0))
                SUB.append((f"qk{j}", (lambda j=j: qk_fin(j, 1)), 0))

            def add_v_block(t):
                for c in range(DC):
                    SUB.append((None, (lambda t=t, c=c: v_mm(t, c)), 240))
                SUB.append((f"v{t}", (lambda t=t: v_fin(t)), 0))

            for blk in ["qk1", "qk5", "v2", "v3",
                        "v4", "qk2", "qk6", "v5", "v6", "v7", "qk3", "qk7"]:
                if blk.startswith("qk"):
                    add_qk_block(int(blk[2:]))
                else:
                    add_v_block(int(blk[1:]))

            total_sub_cost = sum(c for _, _, c in SUB)
            done_labels: set = set()
            sub_pos = [0]
            sub_spent = [0]

            def emit_next_sub():
                label, fn, cost = SUB[sub_pos[0]]
                fn()
                if label is not None:
                    done_labels.add(label)
                sub_pos[0] += 1
                sub_spent[0] += cost
                return cost

            def force(label):
                while label not in done_labels:
                    emit_next_sub()

            def filler_budget(ns):
                spent = 0
                while spent < ns and sub_pos[0] < len(SUB):
                    spent += emit_next_sub()

            # --- phase 2: attention ----------------------------------------
            # Per pair, A-items (kc, half) in this order; B-items follow one
            # pair behind in the same order.
            ORDER = (
                [(k, 0) for k in range(4)]
                + [(k, 0) for k in range(4, 8)]
                + [(k, 1) for k in range(4)]
                + [(k, 1) for k in range(4, 8)]
            )
            ebs: dict[tuple, object] = {}
            ets: dict[tuple, object] = {}
            av_ps: dict[int, object] = {}
            dn_ps: dict[int, object] = {}

            def eb_fetch(pair, kc, half):
                t = ebp.tile([128, S], BF16, tag="eb", name=f"eb{pair}_{kc}_{half}")
                nc.sync.dma_start(out=t[:], in_=eb[pair, kc, half])
                ebs[(pair, kc, half)] = t

            def a_item(pair, kc, half):
                qT = qk_sb[pair]
                kT = qk_sb[4 + pair]
                sc = ps_sc.tile([128, S], F32, tag="sc", name=f"s{pair}_{kc}_{half}")
                for hh in range(2):  # head hh of the pair -> cols hh*512
                    p0 = hh * 64
                    nc.tensor.matmul(
                        sc[:, hh * 512 : (hh + 1) * 512],
                        kT[p0 : p0 + 64, kc * 128 : (kc + 1) * 128],
                        qT[p0 : p0 + 64, half * 512 : (half + 1) * 512],
                        start=True,
                        stop=True,
                    )
                et = ep.tile([128, S], BF16, tag="et", name=f"et{pair}_{kc}_{half}")
                nc.scalar.activation(et[:], sc[:], mybir.ActivationFunctionType.Exp)
                ebt = ebs.pop((pair, kc, half))
                nc.vector.tensor_tensor(et[:], et[:], ebt[:], op=mybir.AluOpType.mult)
                ets[(pair, kc, half)] = et

            def b_item(pair, kc, half, first):
                force(f"v{kc}")
                if kc == 0:
                    av_ps[(pair, half)] = ps_av.tile(
                        [128, 512], F32, tag=f"avH{half}", name=f"av{pair}_{half}"
                    )
                    dn_ps[(pair, half)] = ps_dn.tile(
                        [128, 512], F32, tag=f"dnH{half}", name=f"dn{pair}_{half}"
                    )
                av = av_ps[(pair, half)]
                dn = dn_ps[(pair, half)]
                et = ets.pop((pair, kc, half))
                for hh in range(2):
                    h = 2 * pair + hh
                    nc.tensor.matmul(
                        av[hh * 64 : (hh + 1) * 64, :],
                        v_sb[kc][:, h, :],
                        et[:, hh * 512 : (hh + 1) * 512],
                        start=(kc == 0),
                        stop=(kc == KC - 1),
                    )
                for hh in range(2):
                    drow = hh * 32
                    nc.tensor.matmul(
                        dn[drow : drow + 1, :],
                        ones_col[:],
                        et[:, hh * 512 : (hh + 1) * 512],
                        start=(kc == 0),
                        stop=(kc == KC - 1),
                    )

            def emit_tail_half(pair, half, last):
                av = av_ps.pop((pair, half))
                dn = dn_ps.pop((pair, half))
                cs = slice(half * 512, (half + 1) * 512)
                for hh in range(2):
                    h = 2 * pair + hh
                    drow = hh * 32
                    smf = op_.tile([1, 512], F32, tag="smf")
                    nc.scalar.activation(
                        smf[:], dn[drow : drow + 1, :],
                        mybir.ActivationFunctionType.Copy,
                    )
                    rcf = op_.tile([1, 512], F32, tag="rcf")
                    nc.vector.reciprocal_approx_fast(rcf[:], smf[:])
                    rb = op_.tile([HD, 512], F32, tag="rb")
                    nc.gpsimd.partition_broadcast(rb[:], rcf[:])
                    ot = op_.tile([HD, 512], BF16, tag="ot")
                    nc.vector.tensor_tensor(
                        ot[:], av[hh * 64 : (hh + 1) * 64, :], rb[:],
                        op=mybir.AluOpType.mult,
                    )
                    nc.sync.dma_start(
                        out=oT[h * HD : (h + 1) * HD, cs], in_=ot[:]
                    )

            b_queue: list = []

            def b_step():
                if not b_queue:
                    return
                pair, (kc, half), first, last = b_queue.pop(0)
                b_item(pair, kc, half, first)
                if kc == KC - 1:
                    emit_tail_half(pair, half, last)

            # --- prelude: qk0 + qk4 chunk-major so each arriving hw chunk
            # feeds 4 back-to-back matmuls (keeps the PE dense + HAM warm).
            # qk0 borrows the (idle) AV PSUM banks; qk4 uses the qkv pool.
            qk_ps[(0, 0)] = ps_av.tile(
                [128, 512], F32, tag="avH0", name="pre_qk0a"
            )[:]
            qk_ps[(0, 1)] = ps_av.tile(
                [128, 512], F32, tag="avH1", name="pre_qk0b"
            )[:]
            qk_ps[(4, 0)] = ps_qkv.tile([128, 512], F32, tag="pA", name="pre_qk4a")
            qk_ps[(4, 1)] = ps_qkv.tile([128, 512], F32, tag="pB", name="pre_qk4b")
            pre_v = ps_sc.tile([128, S], F32, tag="sc", name="pre_v01")
            v_ps[0] = pre_v[:, 0:512]
            v_ps[1] = pre_v[:, 512:1024]
            for c in range(DC):
                for j in (0, 4):
                    for half in range(2):
                        nc.tensor.matmul(
                            qk_ps[(j, half)],
                            wT_sb[c][:, j * 128 : (j + 1) * 128],
                            hT_sb[c][:, half * 512 : (half + 1) * 512],
                            start=(c == 0),
                            stop=(c == DC - 1),
                        )
                for t in (0, 1):
                    nc.tensor.matmul(
                        v_ps[t],
                        hT_sb[c][:, t * 128 : (t + 1) * 128],
                        wT_sb[c][:, 2 * HPC * HD : 3 * HPC * HD],
                        start=(c == 0),
                        stop=(c == DC - 1),
                    )
            for j in (0, 4):
                for half in range(2):
                    qk_fin(j, half)
            for t in (0, 1):
                v_fin(t)
            done_labels.add("qk0")
            done_labels.add("qk4")
            done_labels.add("v0")
            done_labels.add("v1")

            NI = NPAIR * len(ORDER)  # 64
            n_done = 0
            for pair in range(NPAIR):
                for oi, (kc, half) in enumerate(ORDER):
                    if oi == 0:
                        force(f"qk{pair}")
                        force(f"qk{4 + pair}")
                        for pf in range(EB_PREFETCH):
                            eb_fetch(pair, *ORDER[pf])
                    if oi + EB_PREFETCH < 16:
                        eb_fetch(pair, *ORDER[oi + EB_PREFETCH])
                    a_item(pair, kc, half)
                    b_step()
                    n_done += 1
                    # adaptive filler: spread remaining qkv over remaining items
                    rem_items = NI - n_done
                    rem_cost = total_sub_cost - sub_spent[0]
                    if rem_items > 0 and rem_cost > 0:
                        filler_budget(min(1500, rem_cost // rem_items + 1))
                b_queue.extend(
                    (pair, item, i == 0, i == len(ORDER) - 1)
                    for i, item in enumerate(ORDER)
                )
            while b_queue:
                b_step()
            while sub_pos[0] < len(SUB):
                emit_next_sub()

    nc.finalize()
    return nc


def shard_inputs(hidden_states, bias, Wqkv_w, Wqkv_b):
    """Slice + lay out the full inputs into 8 per-core input maps."""
    import ml_dtypes

    bf16 = ml_dtypes.bfloat16
    hidden_states = np.asarray(hidden_states, dtype=np.float32)
    bias = np.asarray(bias, dtype=np.float32)
    Wqkv_w = np.asarray(Wqkv_w, dtype=np.float32)
    Wqkv_b = np.asarray(Wqkv_b, dtype=np.float32)

    expb = np.exp(bias)  # (B, H, S, S) fp32

    in_maps = []
    for c in range(N_CORES):
        b, hs = c // 2, (c % 2) * HPC
        rows = np.concatenate(
            [np.arange(g * D + hs * HD, g * D + (hs + HPC) * HD) for g in range(3)]
        )
        wb2 = Wqkv_b[rows][None, :].astype(bf16)
        wbp2 = np.ascontiguousarray(
            Wqkv_b[rows].reshape(12, 128).T
        ).astype(np.float32)
        # eb[pair, kc, half, 128, 1024]: cols 0:512 = head hs+2*pair,
        # cols 512:1024 = head hs+2*pair+1; [k in chunk kc] x [q in half].
        ebt = np.empty((NPAIR, KC, 2, 128, S), dtype=bf16)
        for pair in range(NPAIR):
            for hh in range(2):
                h = hs + 2 * pair + hh
                e = expb[b, h].T.astype(bf16)  # [k, q]
                er = e.reshape(KC, 128, 2, 512)
                for kc in range(KC):
                    for half in range(2):
                        ebt[pair, kc, half, :, hh * 512 : (hh + 1) * 512] = er[
                            kc, :, half
                        ]
        in_maps.append(
            {
                "hw": np.concatenate(
                    [hidden_states[b].T, Wqkv_w[rows].T], axis=1
                ).astype(bf16),
                "wb": wb2,
                "wbp": wbp2,
                "eb": ebt,
            }
        )
    return in_maps


_CACHED_NC = None


def kernel(hidden_states, bias, Wqkv_w, Wqkv_b):
    from concourse.bass_utils import run_bass_kernel_spmd

    global _CACHED_NC
    if _CACHED_NC is None:
        _CACHED_NC = build_bass()
    in_maps = shard_inputs(hidden_states, bias, Wqkv_w, Wqkv_b)
    res = run_bass_kernel_spmd(_CACHED_NC, in_maps, core_ids=list(range(N_CORES)))
    out = np.empty((B, S, D), dtype=np.float32)
    for c in range(N_CORES):
        b, hs = c // 2, (c % 2) * HPC
        out[b, :, hs * HD : (hs + HPC) * HD] = res.results[c]["oT"].T
    return out
